# revision 7
# baseline (speedup 1.0000x reference)
"""GQA attention kernel for 8 Trainium2 NeuronCores.

Sharding (tensor-parallel on heads): core c owns KV head c and Q heads
4c..4c+3.  Wq/Wk/Wv are split column-wise (output dims), Wo row-wise; each
core computes a partial output projection and the host sums the 8 partials.

Device-side layout tricks:
  - hs is pre-transposed on the host to hsT [H, T] so all projections
    contract over the partition dim with clean DMAs.
  - Scores are computed transposed, ST[j, i] (keys on partitions), so the
    attention-prob matrix feeds the A@V matmul directly as the moving
    operand (no on-chip transpose of probs).  Softmax denominators come for
    free from a ones-column appended to V.  Scores are ~N(0,1) so exp needs
    no max-subtraction in fp32.
  - All matmuls run in float32r (full PE rate at free dim >= 256).
"""

import sys

for p in ("/opt/trn_rl_repo",):
    if p not in sys.path:
        sys.path.insert(0, p)

from contextlib import ExitStack

import numpy as np
from einops import rearrange as ein

import concourse.bass as bass
import concourse.mybir as mybir
import concourse.tile as tile
from concourse import bacc
from concourse.bass import ds, ts
from concourse.bass_utils import run_bass_kernel_spmd

F32 = mybir.dt.float32
F32R = mybir.dt.float32r
AF = mybir.ActivationFunctionType

N_CORES = 8
B, S, H = 2, 2048, 2048
NH, NKV, D = 32, 8, 64
HPC = NH // N_CORES        # 4 q heads per core
T = B * S                  # 4096 tokens
TT = 512                   # projection token tile
NT = T // TT               # 8
ITILE = 512                # attention query tile
NI = S // ITILE            # 4 per batch
NJC = S // 128             # 16 key chunks per batch
KC = H // 128              # 16 hidden chunks
SCALE = D ** -0.5


def build_nc():
    nc = bacc.Bacc(None, target_bir_lowering=False)

    hsT = nc.dram_tensor("hsT", [H, T], F32R, kind="ExternalInput")
    wq = nc.dram_tensor("wq", [2, KC, 128, 128], F32R, kind="ExternalInput")
    wkv = nc.dram_tensor("wkv", [KC, 128, 128], F32R, kind="ExternalInput")
    wo = nc.dram_tensor("wo", [2, KC, 128, 128], F32R, kind="ExternalInput")
    bq = nc.dram_tensor("bq", [128, 2], F32, kind="ExternalInput")
    bkv = nc.dram_tensor("bkv", [128, 1], F32, kind="ExternalInput")
    id64d = nc.dram_tensor("id64d", [128, 64], F32R, kind="ExternalInput")
    ones = nc.dram_tensor("ones", [128, 128], F32R, kind="ExternalInput")
    out = nc.dram_tensor("out", [H, T], F32, kind="ExternalOutput")

    with tile.TileContext(nc) as tc, ExitStack() as ctx:
        singles = ctx.enter_context(tc.tile_pool(name="singles", bufs=1))

        wq_sb = singles.tile([128, 2, KC, 128], F32R)
        nc.sync.dma_start(out=wq_sb[:], in_=wq[:].rearrange("a b p f -> p a b f"))
        wkv_sb = singles.tile([128, KC, 128], F32R)
        nc.sync.dma_start(out=wkv_sb[:], in_=wkv[:].rearrange("b p f -> p b f"))
        wo_sb = singles.tile([128, 2, KC, 128], F32R)
        nc.sync.dma_start(out=wo_sb[:], in_=wo[:].rearrange("a b p f -> p a b f"))
        bq_sb = singles.tile([128, 2], F32)
        nc.sync.dma_start(out=bq_sb[:], in_=bq[:])
        bkv_sb = singles.tile([128, 1], F32)
        nc.sync.dma_start(out=bkv_sb[:], in_=bkv[:])
        id_sb = singles.tile([128, 64], F32R)
        nc.sync.dma_start(out=id_sb[:], in_=id64d[:])
        ones_sb = singles.tile([128, 128], F32R)
        nc.sync.dma_start(out=ones_sb[:], in_=ones[:])

        # persistent activations
        qt_sb = singles.tile([128, 2, T], F32R)    # QT: row = q dim (2x128), col = token
        kvt_sb = singles.tile([128, T], F32R)      # rows 0-63 KT, 64-127 VT
        ktd_sb = singles.tile([128, T], F32R)      # rows 64-127 = KT copy (for odd heads)
        vaug_sb = singles.tile([128, B, NJC, 65], F32R)  # V[j, d] chunks + ones col

        hsT_r = hsT[:].rearrange("(c p) n -> p c n", p=128)

        # ---- phase B: projections ----
        with (
            tc.tile_pool(name="hst", bufs=2) as hst_pool,
            tc.tile_pool(name="pj_ps", bufs=3, space="PSUM") as pj_ps,
        ):
            for t in range(NT):
                hst = hst_pool.tile([128, KC, TT], F32R)
                nc.sync.dma_start(out=hst[:], in_=hsT_r[:, :, ts(t, TT)])
                kvp = pj_ps.tile([128, TT], F32)
                for k in range(KC):
                    nc.tensor.matmul(
                        kvp[:], wkv_sb[:, k, :], hst[:, k, :],
                        start=(k == 0), stop=(k == KC - 1),
                    )
                nc.scalar.activation(
                    kvt_sb[:, ts(t, TT)], kvp[:], AF.Identity, bias=bkv_sb[:, 0:1]
                )
                for mc in range(2):
                    qp = pj_ps.tile([128, TT], F32)
                    for k in range(KC):
                        nc.tensor.matmul(
                            qp[:], wq_sb[:, mc, k, :], hst[:, k, :],
                            start=(k == 0), stop=(k == KC - 1),
                        )
                    # out = q*scale + bq*scale  (bq prescaled on host)
                    nc.scalar.activation(
                        qt_sb[:, mc, ts(t, TT)], qp[:], AF.Identity,
                        bias=bq_sb[:, mc:mc + 1], scale=SCALE,
                    )

        # duplicate KT into partitions 64-127 for odd-head score matmuls
        nc.sync.dma_start(out=ktd_sb[64:128, :], in_=kvt_sb[0:64, :])

        # ---- phase C: transpose V to [token, d] chunks, append ones col ----
        with tc.tile_pool(name="vt_ps", bufs=2, space="PSUM") as vt_ps:
            # fill the ones-column via strided DMA (memset fails the f32r
            # ISA check; DMA keeps the f32r producer tag)
            nc.sync.dma_start(
                out=vaug_sb[:, :, :, 64],
                in_=ones[:, 0:B * NJC].rearrange("p (b j) -> p b j", b=B),
            )
            for b in range(B):
                for jc in range(NJC):
                    vp = vt_ps.tile([128, 64], F32R)
                    nc.tensor.transpose(
                        vp[:],
                        kvt_sb[64:128, ds(b * S + jc * 128, 128)],
                        id_sb[64:128, :],
                        tile_position=(64, 0),
                    )
                    nc.vector.tensor_copy(vaug_sb[:, b, jc, 0:64], vp[:])

        # ---- phase D/E: attention + output projection, per query tile ----
        with (
            tc.tile_pool(name="est", bufs=4) as est_pool,
            tc.tile_pool(name="ctx", bufs=3) as ctx_pool,
            tc.tile_pool(name="so", bufs=4) as so_pool,
            tc.tile_pool(name="outp", bufs=3) as out_pool,
            tc.tile_pool(name="st_ps", bufs=3, space="PSUM") as st_ps,
            tc.tile_pool(name="ot_ps", bufs=2, space="PSUM") as ot_ps,
            tc.tile_pool(name="bc_ps", bufs=1, space="PSUM") as bc_ps,
            tc.tile_pool(name="pt_ps", bufs=2, space="PSUM") as pt_ps,
        ):
            for b in range(B):
                for i in range(NI):
                    isl = ds(b * S + i * ITILE, ITILE)
                    ctx_t = ctx_pool.tile([128, 2, ITILE], F32R)
                    for h in range(HPC):
                        mc, lo = h // 2, (h % 2) * 64
                        otp = ot_ps.tile([65, ITILE], F32)
                        pend = None  # software-pipeline AV one step behind ST/exp
                        for jc in range(NJC):
                            stp = st_ps.tile([128, ITILE], F32)
                            jsl = ds(b * S + jc * 128, 128)
                            if lo == 0:
                                nc.tensor.matmul(
                                    stp[:], kvt_sb[0:64, jsl],
                                    qt_sb[0:64, mc, isl], start=True, stop=True,
                                )
                            else:
                                nc.tensor.matmul(
                                    stp[:], ktd_sb[64:128, jsl],
                                    qt_sb[64:128, mc, isl], start=True, stop=True,
                                    tile_position=(64, 0),
                                )
                            est = est_pool.tile([128, ITILE], F32R)
                            nc.scalar.activation(est[:], stp[:], AF.Exp)
                            if pend is not None:
                                pj, pe = pend
                                nc.tensor.matmul(
                                    otp[:], vaug_sb[:, b, pj, :], pe[:],
                                    start=(pj == 0), stop=False,
                                )
                            pend = (jc, est)
                        pj, pe = pend
                        nc.tensor.matmul(
                            otp[:], vaug_sb[:, b, pj, :], pe[:],
                            start=False, stop=True,
                        )
                        # drain OT+sums to SBUF, then reciprocal -> broadcast
                        # (PE) -> scale.  The mul may read at most one PSUM
                        # operand, so OT comes from SBUF, broadcast from PSUM.
                        so = so_pool.tile([65, ITILE], F32R)
                        nc.vector.tensor_copy(so[:], otp[:])
                        with nc.allow_low_precision(reason="f32r tag, fp32 bits"):
                            nc.vector.reciprocal(so[64:65, :], so[64:65, :])
                        bcp = bc_ps.tile([128, ITILE], F32)
                        nc.tensor.matmul(
                            bcp[:], ones_sb[64:65, :], so[64:65, :],
                            start=True, stop=True, tile_position=(64, 0),
                        )
                        nc.vector.tensor_mul(
                            ctx_t[lo:lo + 64, mc, :], so[0:64, :], bcp[0:64, :]
                        )
                    # Wo partial for this query tile
                    for ec in range(KC):
                        ptp = pt_ps.tile([128, ITILE], F32)
                        for hc in range(2):
                            nc.tensor.matmul(
                                ptp[:], wo_sb[:, hc, ec, :], ctx_t[:, hc, :],
                                start=(hc == 0), stop=(hc == 1),
                            )
                        osb = out_pool.tile([128, ITILE], F32)
                        nc.vector.tensor_copy(osb[:], ptp[:])
                        nc.sync.dma_start(out=out[ts(ec, 128), isl], in_=osb[:])

    nc.compile()
    return nc


def _prep_inputs(hidden_states, Wq, bq, Wk, bk, Wv, bv, Wo, bo):
    hs = np.asarray(hidden_states, dtype=np.float32).reshape(T, H)
    hsT = np.ascontiguousarray(hs.T)
    id64d = np.tile(np.eye(64, dtype=np.float32), (2, 1))
    ones = np.ones((128, 128), dtype=np.float32)
    in_maps = []
    for c in range(N_CORES):
        Wq_c = np.asarray(Wq[c * 256:(c + 1) * 256], dtype=np.float32)
        Wk_c = np.asarray(Wk[c * 64:(c + 1) * 64], dtype=np.float32)
        Wv_c = np.asarray(Wv[c * 64:(c + 1) * 64], dtype=np.float32)
        WoS = np.asarray(Wo[:, c * 256:(c + 1) * 256], dtype=np.float32)
        wq_t = np.ascontiguousarray(
            ein(Wq_c, "(a m) (b k) -> a b k m", m=128, k=128))
        wkv_t = np.ascontiguousarray(
            ein(np.concatenate([Wk_c, Wv_c], 0), "m (b k) -> b k m", k=128))
        wo_t = np.ascontiguousarray(
            ein(WoS, "(b m) (a k) -> a b k m", m=128, k=128))
        bq_t = np.ascontiguousarray(
            (np.asarray(bq[c * 256:(c + 1) * 256], dtype=np.float32) * SCALE)
            .reshape(2, 128).T)
        bkv_t = np.concatenate(
            [np.asarray(bk[c * 64:(c + 1) * 64], dtype=np.float32),
             np.asarray(bv[c * 64:(c + 1) * 64], dtype=np.float32)]
        ).reshape(128, 1)
        in_maps.append({
            "hsT": hsT, "wq": wq_t, "wkv": wkv_t, "wo": wo_t,
            "bq": bq_t, "bkv": bkv_t, "id64d": id64d, "ones": ones,
        })
    return in_maps


_NC_CACHE = []


def bench(hidden_states, Wq, bq, Wk, bk, Wv, bv, Wo, bo, iters=10):
    """Time repeated kernel dispatches with device-held inputs; returns min ns.

    NTFF profiling is unavailable in this image, so this is the HW timing
    proxy: inputs/outputs stay on device, each call is one 8-core NEFF
    execution; min over iters strips most of the dispatch jitter.
    """
    import time

    import jax
    from jax.experimental.shard_map import shard_map
    from jax.sharding import Mesh, NamedSharding, PartitionSpec

    import concourse.bass2jax as b2j

    if not _NC_CACHE:
        _NC_CACHE.append(build_nc())
    nc = _NC_CACHE[0]
    in_maps = _prep_inputs(hidden_states, Wq, bq, Wk, bk, Wv, bv, Wo, bo)
    b2j.install_neuronx_cc_hook()

    partition_name = nc.partition_id_tensor.name if nc.partition_id_tensor else None
    in_names, out_names, out_avals, zero_outs = [], [], [], []
    for alloc in nc.m.functions[0].allocations:
        if not isinstance(alloc, mybir.MemoryLocationSet):
            continue
        name = alloc.memorylocations[0].name
        if alloc.kind == "ExternalInput":
            if name != partition_name:
                in_names.append(name)
        elif alloc.kind == "ExternalOutput":
            out_names.append(name)
            shape = tuple(alloc.tensor_shape)
            dtype = mybir.dt.np(alloc.dtype)
            out_avals.append(jax.core.ShapedArray(shape, dtype))
            zero_outs.append(np.zeros(shape, dtype))
    n_params = len(in_names)
    in_names = in_names + out_names
    if partition_name:
        in_names.append(partition_name)

    def _body(*args):
        operands = list(args)
        if partition_name:
            operands.append(b2j.partition_id_tensor())
        outs = b2j._bass_exec_p.bind(
            *operands,
            out_avals=tuple(out_avals),
            in_names=tuple(in_names),
            out_names=tuple(out_names),
            lowering_input_output_aliases=(),
            sim_require_finite=True,
            sim_require_nnan=True,
            nc=nc,
        )
        return tuple(outs)

    devices = jax.devices()[:N_CORES]
    mesh = Mesh(np.asarray(devices), ("core",))
    in_specs = (PartitionSpec("core"),) * (n_params + len(out_names))
    out_specs = (PartitionSpec("core"),) * len(out_names)
    fn = jax.jit(
        shard_map(_body, mesh=mesh, in_specs=in_specs,
                  out_specs=out_specs, check_rep=False),
        keep_unused=True,
    )
    sh = NamedSharding(mesh, PartitionSpec("core"))
    concat_in = [
        np.concatenate([np.asarray(in_maps[c][nm]) for c in range(N_CORES)], 0)
        for nm in in_names[:n_params]
    ]
    concat_zero = [np.zeros((N_CORES * z.shape[0], *z.shape[1:]), z.dtype)
                   for z in zero_outs]
    dev_in = [jax.device_put(a, sh) for a in concat_in + concat_zero]
    r = fn(*dev_in)
    jax.block_until_ready(r)
    times = []
    for _ in range(iters):
        t0 = time.perf_counter()
        r = fn(*dev_in)
        jax.block_until_ready(r)
        times.append(time.perf_counter() - t0)
    bench.times = times
    return min(times) * 1e9


def kernel(hidden_states, Wq, bq, Wk, bk, Wv, bv, Wo, bo, trace=False, **kw):
    if not _NC_CACHE:
        _NC_CACHE.append(build_nc())
    nc = _NC_CACHE[0]
    in_maps = _prep_inputs(hidden_states, Wq, bq, Wk, bk, Wv, bv, Wo, bo)
    res = run_bass_kernel_spmd(nc, in_maps, list(range(N_CORES)), trace=trace, **kw)
    acc = res.results[0]["out"].astype(np.float32)
    for c in range(1, N_CORES):
        acc = acc + res.results[c]["out"]
    outp = acc.T + np.asarray(bo, dtype=np.float32)[None, :]
    outp = outp.reshape(B, S, H).astype(np.float32)
    if trace:
        kernel.last_exec_time_ns = res.exec_time_ns
        kernel.last_results = res
    return outp


# revision 14
# speedup vs baseline: 2.5431x; 2.5431x over previous
"""GQA attention kernel for 8 Trainium2 NeuronCores.

Sharding (tensor-parallel on heads): core c owns KV head c and Q heads
4c..4c+3.  Wq/Wk/Wv are split column-wise (output dims), Wo row-wise; each
core computes a partial output projection and the host sums the 8 partials.

Device-side layout tricks:
  - hs is pre-transposed on the host to hsT [H, T] so all projections
    contract over the partition dim with clean DMAs.
  - Scores are computed transposed, ST[j, i] (keys on partitions), so the
    attention-prob matrix feeds the A@V matmul directly as the moving
    operand (no on-chip transpose of probs).  Softmax denominators come for
    free from a ones-column appended to V.  Scores are ~N(0,1) so exp needs
    no max-subtraction in fp32.
  - All matmuls run in float32r (full PE rate at free dim >= 256).
"""

import sys

for p in ("/opt/trn_rl_repo",):
    if p not in sys.path:
        sys.path.insert(0, p)

from contextlib import ExitStack

import numpy as np
from einops import rearrange as ein

import concourse.bass as bass
import concourse.mybir as mybir
import concourse.tile as tile
from concourse import bacc
from concourse.bass import ds, ts
from concourse.bass_utils import run_bass_kernel_spmd

F32 = mybir.dt.float32
F32R = mybir.dt.float32r
AF = mybir.ActivationFunctionType

N_CORES = 8
B, S, H = 2, 2048, 2048
NH, NKV, D = 32, 8, 64
HPC = NH // N_CORES        # 4 q heads per core
T = B * S                  # 4096 tokens
TT = 512                   # projection token tile
NT = T // TT               # 8
ITILE = 512                # attention query tile
NI = S // ITILE            # 4 per batch
NJC = S // 128             # 16 key chunks per batch
KC = H // 128              # 16 hidden chunks
SCALE = D ** -0.5


def build_nc(loop_iters=None):
    """loop_iters: if set, wrap the whole body in a hardware loop executing it
    that many times — used only for differential timing (N vs 1)."""
    nc = bacc.Bacc(None, target_bir_lowering=False)

    hsT = nc.dram_tensor("hsT", [H, T], F32R, kind="ExternalInput")
    wq = nc.dram_tensor("wq", [2, KC, 128, 128], F32R, kind="ExternalInput")
    wkv = nc.dram_tensor("wkv", [KC, 128, 128], F32R, kind="ExternalInput")
    wo = nc.dram_tensor("wo", [2, KC, 128, 128], F32R, kind="ExternalInput")
    bq = nc.dram_tensor("bq", [128, 2], F32, kind="ExternalInput")
    bkv = nc.dram_tensor("bkv", [128, 1], F32, kind="ExternalInput")
    id64d = nc.dram_tensor("id64d", [128, 64], F32R, kind="ExternalInput")
    ones = nc.dram_tensor("ones", [128, 128], F32R, kind="ExternalInput")
    out = nc.dram_tensor("out", [H, T], F32, kind="ExternalOutput")

    with tile.TileContext(nc) as tc, ExitStack() as ctx:
        if loop_iters is not None:
            ctx.enter_context(tc.For_i(0, loop_iters, 1))
        singles = ctx.enter_context(tc.tile_pool(name="singles", bufs=1))

        wq_sb = singles.tile([128, 2, KC, 128], F32R)
        nc.sync.dma_start(out=wq_sb[:], in_=wq[:].rearrange("a b p f -> p a b f"))
        wkv_sb = singles.tile([128, KC, 128], F32R)
        nc.sync.dma_start(out=wkv_sb[:], in_=wkv[:].rearrange("b p f -> p b f"))
        wo_sb = singles.tile([128, 2, KC, 128], F32R)
        nc.sync.dma_start(out=wo_sb[:], in_=wo[:].rearrange("a b p f -> p a b f"))
        bq_sb = singles.tile([128, 2], F32)
        nc.sync.dma_start(out=bq_sb[:], in_=bq[:])
        bkv_sb = singles.tile([128, 1], F32)
        nc.sync.dma_start(out=bkv_sb[:], in_=bkv[:])
        id_sb = singles.tile([128, 64], F32R)
        nc.sync.dma_start(out=id_sb[:], in_=id64d[:])

        # persistent activations
        qt_sb = singles.tile([128, 2, T], F32R)    # QT: row = q dim (2x128), col = token
        kvt_sb = singles.tile([128, T], F32R)      # rows 0-63 KT, 64-127 VT
        ktd_sb = singles.tile([128, T], F32R)      # rows 64-127 = KT copy (for odd heads)
        vaug_sb = singles.tile([128, B, NJC, 65], F32R)  # V[j, d] chunks + ones col

        hsT_r = hsT[:].rearrange("(c p) n -> p c n", p=128)

        # ---- phase B: projections (+ V transpose and KT dup, per tile) ----
        # fill the ones-column via strided DMA (memset fails the f32r
        # ISA check; DMA keeps the f32r producer tag)
        nc.sync.dma_start(
            out=vaug_sb[:, :, :, 64],
            in_=ones[:, 0:B * NJC].rearrange("p (b j) -> p b j", b=B),
        )
        with (
            tc.tile_pool(name="hst", bufs=2) as hst_pool,
            tc.tile_pool(name="pj_ps", bufs=3, space="PSUM") as pj_ps,
            tc.tile_pool(name="vt_ps", bufs=2, space="PSUM") as vt_ps,
        ):
            for t in range(NT):
                hst = hst_pool.tile([128, KC, TT], F32R)
                nc.sync.dma_start(out=hst[:], in_=hsT_r[:, :, ts(t, TT)])
                kvp = pj_ps.tile([128, TT], F32)
                for k in range(KC):
                    nc.tensor.matmul(
                        kvp[:], wkv_sb[:, k, :], hst[:, k, :],
                        start=(k == 0), stop=(k == KC - 1),
                    )
                nc.scalar.activation(
                    kvt_sb[:, ts(t, TT)], kvp[:], AF.Identity, bias=bkv_sb[:, 0:1]
                )
                for mc in range(2):
                    qp = pj_ps.tile([128, TT], F32)
                    for k in range(KC):
                        nc.tensor.matmul(
                            qp[:], wq_sb[:, mc, k, :], hst[:, k, :],
                            start=(k == 0), stop=(k == KC - 1),
                        )
                    # out = q*scale + bq*scale  (bq prescaled on host)
                    nc.scalar.activation(
                        qt_sb[:, mc, ts(t, TT)], qp[:], AF.Identity,
                        bias=bq_sb[:, mc:mc + 1], scale=SCALE,
                    )
                # KT dup (odd-head score matmuls read it at partitions 64-127)
                nc.sync.dma_start(
                    out=ktd_sb[64:128, ts(t, TT)], in_=kvt_sb[0:64, ts(t, TT)]
                )
                # V transpose for this tile's 4 key chunks
                tb, tj0 = t // (NT // B), (t % (NT // B)) * (TT // 128)
                for jo in range(TT // 128):
                    jc = tj0 + jo
                    vp = vt_ps.tile([128, 64], F32R)
                    nc.tensor.transpose(
                        vp[:],
                        kvt_sb[64:128, ds(tb * S + jc * 128, 128)],
                        id_sb[64:128, :],
                        tile_position=(64, 0),
                    )
                    nc.vector.tensor_copy(vaug_sb[:, tb, jc, 0:64], vp[:])

        # ---- phase D/E: attention + output projection, per query tile ----
        with (
            tc.tile_pool(name="est", bufs=3) as est_pool,
            tc.tile_pool(name="ctx", bufs=3) as ctx_pool,
            tc.tile_pool(name="so", bufs=4) as so_pool,
            tc.tile_pool(name="bcs", bufs=4) as bcs_pool,
            tc.tile_pool(name="outp", bufs=3) as out_pool,
            tc.tile_pool(name="st_ps", bufs=2, space="PSUM") as st_ps,
            tc.tile_pool(name="ot_ps", bufs=2, space="PSUM") as ot_ps,
            tc.tile_pool(name="pt_ps", bufs=2, space="PSUM") as pt_ps,
        ):
            NJP = NJC // 2  # key chunks processed in pairs per exp
            for b in range(B):
                for i in range(NI):
                    isl = ds(b * S + i * ITILE, ITILE)
                    ctx_t = ctx_pool.tile([128, 2, ITILE], F32R)
                    for h in range(HPC):
                        mc, lo = h // 2, (h % 2) * 64
                        otp = ot_ps.tile([65, ITILE], F32)
                        pend = None  # software-pipeline AV one step behind ST/exp
                        for jp in range(NJP):
                            stp = st_ps.tile([128, 2, ITILE], F32)
                            for u in range(2):
                                jc = jp * 2 + u
                                jsl = ds(b * S + jc * 128, 128)
                                if lo == 0:
                                    nc.tensor.matmul(
                                        stp[:, u, :], kvt_sb[0:64, jsl],
                                        qt_sb[0:64, mc, isl],
                                        start=True, stop=True,
                                    )
                                else:
                                    nc.tensor.matmul(
                                        stp[:, u, :], ktd_sb[64:128, jsl],
                                        qt_sb[64:128, mc, isl],
                                        start=True, stop=True,
                                        tile_position=(64, 0),
                                    )
                            est = est_pool.tile([128, 2, ITILE], F32R)
                            nc.scalar.activation(est[:], stp[:], AF.Exp)
                            if pend is not None:
                                pj, pe = pend
                                for u in range(2):
                                    nc.tensor.matmul(
                                        otp[:], vaug_sb[:, b, pj * 2 + u, :],
                                        pe[:, u, :],
                                        start=(pj == 0 and u == 0), stop=False,
                                    )
                            pend = (jp, est)
                        pj, pe = pend
                        for u in range(2):
                            nc.tensor.matmul(
                                otp[:], vaug_sb[:, b, pj * 2 + u, :], pe[:, u, :],
                                start=(pj == 0 and u == 0), stop=(u == 1),
                            )
                        # drain OT+sums to SBUF; reciprocal (row 0); GPSIMD
                        # partition-broadcast; scale.  All SBUF, no PE/PSUM.
                        so = so_pool.tile([65, ITILE], F32R)
                        nc.vector.tensor_copy(so[:], otp[:])
                        rec = so_pool.tile([1, ITILE], F32R, tag="rec")
                        with nc.allow_low_precision(reason="f32r tag, fp32 bits"):
                            nc.vector.reciprocal(rec[0:1, :], so[64:65, :])
                        bcs = bcs_pool.tile([64, ITILE], F32R)
                        nc.gpsimd.partition_broadcast(bcs[:], rec[0:1, :])
                        nc.vector.tensor_mul(
                            ctx_t[lo:lo + 64, mc, :], so[0:64, :], bcs[:, :]
                        )
                    # Wo partial for this query tile
                    for ec in range(KC):
                        ptp = pt_ps.tile([128, ITILE], F32)
                        for hc in range(2):
                            nc.tensor.matmul(
                                ptp[:], wo_sb[:, hc, ec, :], ctx_t[:, hc, :],
                                start=(hc == 0), stop=(hc == 1),
                            )
                        osb = out_pool.tile([128, ITILE], F32)
                        nc.vector.tensor_copy(osb[:], ptp[:])
                        nc.sync.dma_start(out=out[ts(ec, 128), isl], in_=osb[:])

    nc.compile()
    return nc


def _prep_inputs(hidden_states, Wq, bq, Wk, bk, Wv, bv, Wo, bo):
    hs = np.asarray(hidden_states, dtype=np.float32).reshape(T, H)
    hsT = np.ascontiguousarray(hs.T)
    id64d = np.tile(np.eye(64, dtype=np.float32), (2, 1))
    ones = np.ones((128, 128), dtype=np.float32)
    in_maps = []
    for c in range(N_CORES):
        Wq_c = np.asarray(Wq[c * 256:(c + 1) * 256], dtype=np.float32)
        Wk_c = np.asarray(Wk[c * 64:(c + 1) * 64], dtype=np.float32)
        Wv_c = np.asarray(Wv[c * 64:(c + 1) * 64], dtype=np.float32)
        WoS = np.asarray(Wo[:, c * 256:(c + 1) * 256], dtype=np.float32)
        wq_t = np.ascontiguousarray(
            ein(Wq_c, "(a m) (b k) -> a b k m", m=128, k=128))
        wkv_t = np.ascontiguousarray(
            ein(np.concatenate([Wk_c, Wv_c], 0), "m (b k) -> b k m", k=128))
        wo_t = np.ascontiguousarray(
            ein(WoS, "(b m) (a k) -> a b k m", m=128, k=128))
        bq_t = np.ascontiguousarray(
            (np.asarray(bq[c * 256:(c + 1) * 256], dtype=np.float32) * SCALE)
            .reshape(2, 128).T)
        bkv_t = np.concatenate(
            [np.asarray(bk[c * 64:(c + 1) * 64], dtype=np.float32),
             np.asarray(bv[c * 64:(c + 1) * 64], dtype=np.float32)]
        ).reshape(128, 1)
        in_maps.append({
            "hsT": hsT, "wq": wq_t, "wkv": wkv_t, "wo": wo_t,
            "bq": bq_t, "bkv": bkv_t, "id64d": id64d, "ones": ones,
        })
    return in_maps


_NC_CACHE = []


def bench(hidden_states, Wq, bq, Wk, bk, Wv, bv, Wo, bo, iters=10):
    """Time repeated kernel dispatches with device-held inputs; returns min ns.

    NTFF profiling is unavailable in this image, so this is the HW timing
    proxy: inputs/outputs stay on device, each call is one 8-core NEFF
    execution; min over iters strips most of the dispatch jitter.
    """
    import time

    import jax
    from jax.experimental.shard_map import shard_map
    from jax.sharding import Mesh, NamedSharding, PartitionSpec

    import concourse.bass2jax as b2j

    if not _NC_CACHE:
        _NC_CACHE.append(build_nc())
    nc = _NC_CACHE[0]
    in_maps = _prep_inputs(hidden_states, Wq, bq, Wk, bk, Wv, bv, Wo, bo)
    b2j.install_neuronx_cc_hook()

    partition_name = nc.partition_id_tensor.name if nc.partition_id_tensor else None
    in_names, out_names, out_avals, zero_outs = [], [], [], []
    for alloc in nc.m.functions[0].allocations:
        if not isinstance(alloc, mybir.MemoryLocationSet):
            continue
        name = alloc.memorylocations[0].name
        if alloc.kind == "ExternalInput":
            if name != partition_name:
                in_names.append(name)
        elif alloc.kind == "ExternalOutput":
            out_names.append(name)
            shape = tuple(alloc.tensor_shape)
            dtype = mybir.dt.np(alloc.dtype)
            out_avals.append(jax.core.ShapedArray(shape, dtype))
            zero_outs.append(np.zeros(shape, dtype))
    n_params = len(in_names)
    in_names = in_names + out_names
    if partition_name:
        in_names.append(partition_name)

    def _body(*args):
        operands = list(args)
        if partition_name:
            operands.append(b2j.partition_id_tensor())
        outs = b2j._bass_exec_p.bind(
            *operands,
            out_avals=tuple(out_avals),
            in_names=tuple(in_names),
            out_names=tuple(out_names),
            lowering_input_output_aliases=(),
            sim_require_finite=True,
            sim_require_nnan=True,
            nc=nc,
        )
        return tuple(outs)

    devices = jax.devices()[:N_CORES]
    mesh = Mesh(np.asarray(devices), ("core",))
    in_specs = (PartitionSpec("core"),) * (n_params + len(out_names))
    out_specs = (PartitionSpec("core"),) * len(out_names)
    fn = jax.jit(
        shard_map(_body, mesh=mesh, in_specs=in_specs,
                  out_specs=out_specs, check_rep=False),
        keep_unused=True,
    )
    sh = NamedSharding(mesh, PartitionSpec("core"))
    concat_in = [
        np.concatenate([np.asarray(in_maps[c][nm]) for c in range(N_CORES)], 0)
        for nm in in_names[:n_params]
    ]
    concat_zero = [np.zeros((N_CORES * z.shape[0], *z.shape[1:]), z.dtype)
                   for z in zero_outs]
    dev_in = [jax.device_put(a, sh) for a in concat_in + concat_zero]
    r = fn(*dev_in)
    jax.block_until_ready(r)
    times = []
    for _ in range(iters):
        t0 = time.perf_counter()
        r = fn(*dev_in)
        jax.block_until_ready(r)
        times.append(time.perf_counter() - t0)
    bench.times = times
    return min(times) * 1e9


def kernel(hidden_states, Wq, bq, Wk, bk, Wv, bv, Wo, bo, trace=False, **kw):
    if not _NC_CACHE:
        _NC_CACHE.append(build_nc())
    nc = _NC_CACHE[0]
    in_maps = _prep_inputs(hidden_states, Wq, bq, Wk, bk, Wv, bv, Wo, bo)
    res = run_bass_kernel_spmd(nc, in_maps, list(range(N_CORES)), trace=trace, **kw)
    acc = res.results[0]["out"].astype(np.float32)
    for c in range(1, N_CORES):
        acc = acc + res.results[c]["out"]
    outp = acc.T + np.asarray(bo, dtype=np.float32)[None, :]
    outp = outp.reshape(B, S, H).astype(np.float32)
    if trace:
        kernel.last_exec_time_ns = res.exec_time_ns
        kernel.last_results = res
    return outp
